# revision 17
# baseline (speedup 1.0000x reference)
"""CoordinatesToSpikes on 8 TRN2 NeuronCores.

Reference semantics: times = T_EARLY + cv * (T_LATE - T_EARLY);
idx = round(times / DT); spikes = one-hot along a dense time axis of
length 1000 (each (b, c) pair scatters exactly one 1.0, so the scatter
is a pure one-hot materialization: out[b, t, c] = (idx[b, c] == t)).

Strategy (data-parallel over batch, 256 -> 8 x 32):
  - Host computes idx bit-exactly in fp32 (tiny: 64K elements) and a
    per-core diff tensor diff[p, f] = idx[p%32, f%256] - (p//32)*250
    - f//256 (1.25MB/core); partition p = tg*32 + b covers batch b,
    time-quarter tg. All values are exact small integers.
  - idx <= 800 always (times < 8e-4), so output rows 810..999 are
    guaranteed zeros: they are streamed from a zero tile (stride-0
    repeated source) starting right after the kernel preamble, with no
    data dependency -- filling the HBM window while diff loads.
  - Each of 25 chunks (10 time rows) is one DVE compare diff == 10*d
    producing the one-hot tile [128, 2560]; it is stored per
    time-quarter as [32, 2560] transfers (contiguous partition slices,
    10KB contiguous per partition). Chunks 6..24 skip tg=3 (rows
    810..999, already zero-filled). Stores rotate across three DGE
    queues (2 HWDGE rings + the GpSimd SWDGE ring).
  - Output is write-only, 32.8 MB per core => memory(store)-roofline;
    HBM stacks are shared pairwise (716 GB/s per 2 cores), so
    ~358 GB/s/core sustained and ~91.5us of unavoidable store time.
"""

import numpy as np
from contextlib import ExitStack

import concourse.bass as bass
import concourse.tile as tile
from concourse import bacc, mybir
from concourse.bass_utils import run_bass_kernel_spmd

F32 = mybir.dt.float32

B, C, SEQ = 256, 256, 1000
NCORES = 8
BSH = B // NCORES          # 32 batches per core
TG = 4                     # time quarters (partition = tg*32 + b)
TQ = SEQ // TG             # 250 time rows per quarter
TROWS = 10                 # time rows per chunk
ND = TQ // TROWS           # 25 chunks
FREE = TROWS * C           # 2560 free elements per tile (10KB)
ZROW = 810                 # rows >= ZROW are guaranteed zero (idx <= 800)
NZREP = (SEQ - ZROW) // TROWS  # 19 repeats of the 10-row zero tile

T_EARLY = np.float32(2e-06)
T_LATE_MINUS_EARLY = np.float32(0.0008 - 2e-06)
DT = np.float32(1e-06)

_compiled = None


def _build():
    nc = bacc.Bacc("TRN2", target_bir_lowering=False, debug=False,
                   num_devices=NCORES)
    diff_d = nc.dram_tensor("diff", [128, FREE], F32, kind="ExternalInput")
    out_d = nc.dram_tensor("out", [BSH, SEQ, C], F32, kind="ExternalOutput")

    def rows_ap(t0, nrows):
        # [32 batches (1MB stride), nrows*256 contiguous] view of
        # out[:, t0:t0+nrows, :]
        return out_d.ap()[:, t0:t0 + nrows, :].rearrange("b t c -> b (t c)")

    quart = FREE // 4
    with ExitStack() as ctx:
        tc = ctx.enter_context(tile.TileContext(nc))
        dpool = ctx.enter_context(tc.tile_pool(name="diff", bufs=1))
        zpool = ctx.enter_context(tc.tile_pool(name="zero", bufs=1))
        outp = ctx.enter_context(tc.tile_pool(name="outp", bufs=10))

        engines = [nc.sync, nc.scalar, nc.gpsimd]

        # diff load: four quarters, two per HWDGE ring.
        diff = dpool.tile([128, FREE], F32)
        for q in range(4):
            engines[q % 2].dma_start(
                diff[:, q * quart:(q + 1) * quart],
                diff_d.ap()[:, q * quart:(q + 1) * quart])

        # Zero-prefill rows 810..999: no data dependency, so these
        # stream while diff is still loading. Stride-0 source repeats
        # the 10-row zero tile.
        zero = zpool.tile([BSH, FREE], F32)
        nc.vector.memset(zero[:], 0.0)
        zsplit = [(ZROW, 9), (ZROW + 90, 10)]
        for i, (t0, reps) in enumerate(zsplit):
            engines[i].dma_start(
                out_d.ap()[:, t0:t0 + reps * TROWS, :].rearrange(
                    "b (r t) c -> b r (t c)", t=TROWS),
                zero[:].unsqueeze(1).broadcast_to((BSH, reps, FREE)))

        # Compare chunks; store per quarter (contiguous partition
        # slices). Chunks >= 6 skip tg=3 (rows 810+ already zeroed).
        si = 0
        for d in range(ND):
            ot = outp.tile([128, FREE], F32)
            nc.vector.tensor_scalar(
                ot[:], diff[:], float(TROWS * d), None,
                mybir.AluOpType.is_equal)
            ntg = TG if d < 6 else TG - 1
            for tg in range(ntg):
                engines[si % 3].dma_start(
                    rows_ap(tg * TQ + d * TROWS, TROWS),
                    ot[tg * BSH:(tg + 1) * BSH, :])
                si += 1
    nc.compile()
    return nc


def _host_idx(coordinate_values: np.ndarray) -> np.ndarray:
    """Bit-exact fp32 mirror of the reference index computation."""
    cv = np.ascontiguousarray(coordinate_values, dtype=np.float32)
    times = T_EARLY + cv * T_LATE_MINUS_EARLY
    return np.rint(times / DT).astype(np.float32)


def _in_maps(coordinate_values: np.ndarray) -> list[dict]:
    idxf = _host_idx(coordinate_values)                      # (256, 256)
    p = np.arange(128)
    base = ((p // BSH) * TQ)[:, None] + np.repeat(
        np.arange(TROWS), C)[None, :]                        # (128, 2560)
    maps = []
    for m in range(NCORES):
        shard = idxf[m * BSH:(m + 1) * BSH]                  # (32, 256)
        tiled = np.tile(shard[p % BSH], (1, TROWS))          # (128, 2560)
        maps.append({"diff": (tiled - base).astype(np.float32)})
    return maps


def kernel(coordinate_values: np.ndarray) -> np.ndarray:
    global _compiled
    if _compiled is None:
        _compiled = _build()
    res = run_bass_kernel_spmd(
        _compiled, _in_maps(coordinate_values),
        core_ids=list(range(NCORES)))
    return np.concatenate([r["out"] for r in res.results], axis=0)
